# revision 2
# baseline (speedup 1.0000x reference)
"""Trainium2 Bass kernel for nn_CFI_Module (non-local attention block), fp8.

v2: engine-rebalanced schedule.  Cost-model analysis showed the baseline was
ACT-bound (exp 16.6us + 15us of drains on ACT while DVE idled).  This version
pins ACT to the 16 exps (plus startup/tail shares), routes chain-window PSUM
drains to DVE, merges drains into wider instructions, warms the PE early, and
balances the tail (y/om drains) across both engines.

Math/scales identical to baseline (validated l2 ~ 3.7e-3):
  - attention-path matmuls fp8 DoubleRow; weights pre-scaled 2^6 on host
  - E = exp(scores * 2^-12) in e5m2 with accum Z; GT = g^T * 2^14/Z e4m3
  - y psum *2^-7 -> e4m3;  om staged raw (=2^9 om) e4m3, host *2^-9
  - W_AB skip conv fp8 hi/lo 3-term; OW staged f16 *2^-6
"""
import sys

for _p in ("/opt/trn_rl_repo", "/root/.axon_site/_ro/trn_rl_repo"):
    if _p not in sys.path:
        sys.path.append(_p)

import numpy as np
from contextlib import ExitStack, nullcontext

import ml_dtypes
import concourse.bacc as bacc
import concourse.tile as tile
from concourse import mybir
from concourse.bass_utils import run_bass_kernel_spmd

F32 = mybir.dt.float32
F16 = mybir.dt.float16
E4 = mybir.dt.float8e4
E5 = mybir.dt.float8e5
E4NP = ml_dtypes.float8_e4m3
E5NP = ml_dtypes.float8_e5m2
DR = mybir.MatmulPerfMode.DoubleRow
EXP = mybir.ActivationFunctionType.Exp
MUL = mybir.AluOpType.mult

_NC_CACHE = {}

# resident y blocks: (0,0),(0,1) share the psM [1024] tile; (1,0) in psW
RES_PAIR = [(0, 0), (0, 1)]
RES_W = (1, 0)


def build_nc():
    nc = bacc.Bacc(target_bir_lowering=False, trn_type="TRN2")

    # ---- DRAM I/O ----
    B8_d = nc.dram_tensor("B8", [128, 2, 2, 2048], E4, kind="ExternalInput")
    AH8_d = nc.dram_tensor("AH8", [128, 2, 2048], E4, kind="ExternalInput")
    BH8_d = nc.dram_tensor("BH8", [128, 2, 2048], E4, kind="ExternalInput")
    ALO_d = nc.dram_tensor("ALO", [128, 2, 2048], E4, kind="ExternalInput")
    BLO_d = nc.dram_tensor("BLO", [128, 2, 2048], E4, kind="ExternalInput")
    WQ1_d = nc.dram_tensor("WQ1", [128, 1280], E4, kind="ExternalInput")
    WQ2_d = nc.dram_tensor("WQ2", [128, 3072], E4, kind="ExternalInput")
    # om output: [p, st, och, n];  host: out[och*128+p, st*2048+n] = OM*2^-9
    OM_d = nc.dram_tensor("OM8", [128, 2, 2, 2048], E4, kind="ExternalOutput")
    OW_d = nc.dram_tensor("OW", [2, 128, 2048], F16, kind="ExternalOutput")

    with tile.TileContext(nc) as tc:
        with ExitStack() as ctx:
            io = ctx.enter_context(tc.tile_pool(name="io", bufs=1))
            acts = ctx.enter_context(tc.tile_pool(name="acts", bufs=1))
            spool = ctx.enter_context(tc.tile_pool(name="spool", bufs=8))
            wstg = ctx.enter_context(tc.tile_pool(name="wstg", bufs=2))
            stg = ctx.enter_context(tc.tile_pool(name="stg", bufs=2))
            # PSUM: psS 2x[1024] (4 banks), psM 1x[1024] (2), psG [512] (1),
            # psW [512] (1)  -> 8 banks
            psS = ctx.enter_context(tc.tile_pool(name="psS", bufs=2, space="PSUM"))
            psM = ctx.enter_context(tc.tile_pool(name="psM", bufs=1, space="PSUM"))
            psG = ctx.enter_context(tc.tile_pool(name="psG", bufs=1, space="PSUM"))
            psW = ctx.enter_context(tc.tile_pool(name="psW", bufs=1, space="PSUM"))

            # ---- SBUF ----
            b8 = io.tile([128, 2, 2, 2048], E4, name="b8")
            ah8 = io.tile([128, 2, 2048], E4, name="ah8")
            bh8 = io.tile([128, 2, 2048], E4, name="bh8")
            alo = io.tile([128, 2, 2048], E4, name="alo")
            blo = io.tile([128, 2, 2048], E4, name="blo")
            wq1 = io.tile([128, 10, 128], E4, name="wq1")
            wq2 = io.tile([128, 12, 2, 128], E4, name="wq2")
            scr = io.tile([128, 640], E4, name="scr")
            TH = acts.tile([128, 2, 2048], E4, name="TH")
            PH = acts.tile([128, 2, 1024], E4, name="PH")
            Et = acts.tile([128, 8, 2048], E5, name="Et")
            GT = acts.tile([128, 2, 8, 128], E4, name="GT")
            Y8 = acts.tile([128, 2, 2048], E4, name="Y8")

            wth = wq1[:, 0:2, :]
            wph = wq1[:, 2:4, :]
            wgA = wq1[:, 4:6, :]
            wgB = wq1[:, 6:8, :]

            # ---- PE warmup: memset scratch (Pool) + 4 throwaway matmuls ----
            nc.gpsimd.memset(scr, 0.0)
            wps = psW.tile([128, 512], F32, tag="w", name="wps")
            for _ in range(4):
                nc.tensor.matmul(wps, scr[:, 0:128], scr[:, 128:640],
                                 start=True, stop=True)

            # ---- input DMAs, ordered by first use ----
            # wth+wph first (512B), then theta data, then phi strip cols
            nc.sync.dma_start(out=wq1[:, 0:4, :], in_=WQ1_d[:, 0:512])
            nc.sync.dma_start(out=b8[:, :, 1, 0:1024], in_=B8_d[:, :, 1, 0:1024])
            nc.sync.dma_start(out=b8[:, :, 1, 1024:2048],
                              in_=B8_d[:, :, 1, 1024:2048])
            nc.sync.dma_start(out=ah8[:, :, 0:384], in_=AH8_d[:, :, 0:384])
            nc.sync.dma_start(out=ah8[:, :, 1024:1408],
                              in_=AH8_d[:, :, 1024:1408])
            nc.sync.dma_start(out=wq1[:, 4:10, :], in_=WQ1_d[:, 512:1280])
            nc.sync.dma_start(out=ah8[:, :, 384:1024], in_=AH8_d[:, :, 384:1024])
            nc.sync.dma_start(out=ah8[:, :, 1408:2048],
                              in_=AH8_d[:, :, 1408:2048])
            nc.sync.dma_start(out=b8[:, :, 0, :], in_=B8_d[:, :, 0, :])
            nc.sync.dma_start(out=bh8, in_=BH8_d[:, :, :])
            nc.sync.dma_start(out=wq2, in_=WQ2_d[:, :])
            nc.sync.dma_start(out=alo, in_=ALO_d[:, :, :])
            nc.sync.dma_start(out=blo, in_=BLO_d[:, :, :])

            # ---- theta conv units: (hh, nh) -> TH[:, hh, 1024nh:...] ----
            # engines: one char per 512-drain ("a"/"v"); len 1 = one [1024]
            def theta_u(hh, nh, pool, engines):
                tp = pool.tile([128, 1024], F32,
                               tag="s" if pool is psS else "m", name="tp")
                src = b8[:, :, nh, 1024 * hh:1024 * (hh + 1)]
                with tc.high_priority(offset=800000):
                    for jj in range(2):
                        nc.tensor.matmul(
                            tp[:, 512 * jj:512 * (jj + 1)],
                            wth, src[:, :, 512 * jj:512 * (jj + 1)],
                            start=True, stop=True, perf_mode=DR,
                        )
                    dst = TH[:, hh, 1024 * nh:1024 * (nh + 1)]
                    if len(engines) == 1:
                        if engines == "a":
                            nc.scalar.copy(dst, tp)
                        else:
                            nc.vector.tensor_copy(dst, tp)
                    else:
                        for jj, e in enumerate(engines):
                            d = dst[:, 512 * jj:512 * (jj + 1)]
                            s = tp[:, 512 * jj:512 * (jj + 1)]
                            if e == "a":
                                nc.scalar.copy(d, s)
                            else:
                                nc.vector.tensor_copy(d, s)

            # ---- phi pieces -> PH[:, :, m] (both hh in one drain) ----
            def phi_piece(m0, m1, pool, engine):
                w = m1 - m0
                pp = pool.tile([128, 2, 512] if w > 128 else [128, 2, 128],
                               F32, tag="g" if pool is psG else "m", name="pp")
                with tc.high_priority(offset=800000):
                    for hh in range(2):
                        nc.tensor.matmul(
                            pp[:, hh, 0:w], wph,
                            ah8[:, :, 1024 * hh + m0:1024 * hh + m1],
                            start=True, stop=True, perf_mode=DR,
                        )
                    dst = PH[:, :, m0:m1]
                    if engine == "a":
                        nc.scalar.copy(dst, pp[:, :, 0:w])
                    else:
                        nc.vector.tensor_copy(dst, pp[:, :, 0:w])

            # ---- ow (W_AB skip conv, fp8 hi/lo) ----
            ow_stages = {}

            def ow_job(och, cg):
                fw = psW.tile([128, 512], F32, tag="w", name="fw")
                c0, c1 = 512 * cg, 512 * (cg + 1)
                movers = {
                    0: (ah8[:, :, c0:c1], bh8[:, :, c0:c1]),
                    1: (alo[:, :, c0:c1], blo[:, :, c0:c1]),
                    2: (ah8[:, :, c0:c1], bh8[:, :, c0:c1]),
                }
                first = True
                with tc.high_priority(offset=-400000):
                    for term in range(3):
                        for jstep in range(2):
                            nc.tensor.matmul(
                                fw,
                                wq2[:, term * 4 + och * 2 + jstep, :, :],
                                movers[term][jstep],
                                start=first,
                                stop=(term == 2 and jstep == 1),
                                perf_mode=DR,
                            )
                            first = False
                key = (och, cg // 2)
                if key not in ow_stages:
                    ow_stages[key] = wstg.tile([128, 1024], F16, tag="wst",
                                               name=f"ow{och}_{cg // 2}")
                st_t = ow_stages[key]
                nc.vector.tensor_scalar_mul(
                    st_t[:, 512 * (cg % 2):512 * (cg % 2 + 1)], fw, 2.0 ** -6)
                if cg % 2 == 1:
                    nc.sync.dma_start(
                        out=OW_d[och, :, 1024 * (cg // 2):1024 * (cg // 2 + 1)],
                        in_=st_t,
                    )

            ow_jobs = [(och, cg) for cg in range(4) for och in range(2)]

            # ---- scores + exp ----
            zs = []
            for k in range(8):
                zs.append(spool.tile([128, 4], F32, tag="z", name=f"z{k}"))

            def scores_exp(k, h2):
                sp = psS.tile([128, 1024], F32, tag="s", name="sp")
                with tc.high_priority(offset=800000):
                    for jj in range(2):
                        nc.tensor.matmul(
                            sp[:, 512 * jj:512 * (jj + 1)],
                            PH[:, :, 128 * k:128 * (k + 1)],
                            TH[:, :, 1024 * h2 + 512 * jj:
                               1024 * h2 + 512 * (jj + 1)],
                            start=True, stop=True, perf_mode=DR,
                        )
                    nc.scalar.activation(
                        out=Et[:, k, 1024 * h2:1024 * (h2 + 1)],
                        in_=sp,
                        func=EXP,
                        scale=2.0 ** -12,
                        accum_out=zs[k][:, h2:h2 + 1],
                    )

            # ---- g conv + 1/Z scale -> GT ----
            def gt_k(k):
                with tc.high_priority(offset=700000):
                    nc.vector.tensor_add(zs[k][:, 2:3], zs[k][:, 0:1],
                                         zs[k][:, 1:2])
                    nc.vector.reciprocal(zs[k][:, 3:4], zs[k][:, 2:3])
                gp = psG.tile([128, 512], F32, tag="g", name="gp")
                for st in range(2):
                    with tc.high_priority(offset=-200000):
                        nc.tensor.matmul(
                            gp[:, 128 * st:128 * (st + 1)],
                            ah8[:, :, 1024 * st + 128 * k:1024 * st + 128 * (k + 1)],
                            wgA,
                            start=True, stop=False, perf_mode=DR,
                        )
                        nc.tensor.matmul(
                            gp[:, 128 * st:128 * (st + 1)],
                            bh8[:, :, 1024 * st + 128 * k:
                                1024 * st + 128 * (k + 1)],
                            wgB,
                            start=False, stop=True, perf_mode=DR,
                        )
                with tc.high_priority(offset=700000):
                    nc.vector.tensor_scalar(
                        GT[:, :, k, :], gp[:, 0:256],
                        zs[k][:, 3:4], 256.0, op0=MUL, op1=MUL,
                    )

            # ---- y accumulation ----
            y_ps = {}

            def y_step(st, nb, p):
                with tc.high_priority(offset=-300000):
                    nc.tensor.matmul(
                        y_ps[(st, nb)],
                        GT[:, st, 2 * p:2 * p + 2, :],
                        Et[:, 2 * p:2 * p + 2, 512 * nb:512 * (nb + 1)],
                        start=(p == 0), stop=(p == 3),
                        perf_mode=DR,
                    )

            # ================= schedule =================
            # startup: theta nh=1 units (both hh), phi piece0
            theta_u(0, 1, psS, "av")
            theta_u(1, 1, psS, "av")
            phi_piece(0, 128, psG, "v")

            # sweep 1 (h2=1) with phi/theta/ow fill on DVE
            ow_i = 0
            for k in range(8):
                scores_exp(k, 1)
                if k == 0:
                    phi_piece(128, 384, psM, "v")
                elif k == 1:
                    phi_piece(384, 768, psM, "v")
                elif k == 2:
                    phi_piece(768, 1024, psM, "v")
                elif k == 3:
                    theta_u(0, 0, psM, "v")
                elif k == 4:
                    theta_u(1, 0, psM, "v")
                elif k >= 5:
                    ow_job(*ow_jobs[ow_i])
                    ow_i += 1

            # resident y tiles: psM pair + psW
            resM = psM.tile([128, 1024], F32, tag="m", name="resM")
            y_ps[(0, 0)] = resM[:, 0:512]
            y_ps[(0, 1)] = resM[:, 512:1024]
            resW = psW.tile([128, 512], F32, tag="w", name="resW")
            y_ps[RES_W] = resW

            # sweep 2 (h2=0): gt + resident y ride along; rest of ow early
            for k in range(8):
                scores_exp(k, 0)
                gt_k(k)
                if k < 5 and ow_i < len(ow_jobs):
                    ow_job(*ow_jobs[ow_i])
                    ow_i += 1
                if k % 2 == 1 and k < 7:
                    p = (k - 1) // 2
                    for (st, nb) in (*RES_PAIR, RES_W):
                        y_step(st, nb, p)

            # ---- tail ----
            om_stage = [stg.tile([128, 2, 2048], E4, tag="om", name=f"om{st}")
                        for st in range(2)]
            om_rot = [psS, psS]

            def om_pair(st, nb, engine):
                pool = om_rot.pop(0)
                omp = pool.tile([128, 2, 512], F32,
                                tag="s" if pool is psS else "m", name="omp")
                om_rot.append(pool)
                for och in range(2):
                    nc.tensor.matmul(
                        omp[:, och, :], wq1[:, 8 + och, :],
                        Y8[:, st, 512 * nb:512 * (nb + 1)],
                        start=True, stop=True,
                    )
                dst = om_stage[st][:, :, 512 * nb:512 * (nb + 1)]
                if engine == "a":
                    nc.scalar.copy(dst, omp)
                else:
                    nc.vector.tensor_copy(dst, omp)
                nc.sync.dma_start(
                    out=OM_d[:, st, :, 512 * nb:512 * (nb + 1)],
                    in_=dst,
                )

            def y_drain(st, nb, engine, width=512):
                t = y_ps.pop((st, nb))
                dst = Y8[:, st, 512 * nb:512 * nb + width]
                if engine == "v":
                    nc.vector.tensor_scalar_mul(dst, t, 2.0 ** -7)
                else:
                    nc.scalar.mul(dst, t, 2.0 ** -7)

            # finish resident blocks
            for (st, nb) in (*RES_PAIR, RES_W):
                y_step(st, nb, 3)
            # merged drain of the psM pair on ACT; psW block on DVE
            y_ps.pop((0, 1))
            y_ps[(0, 0)] = resM
            y_drain(0, 0, "a", width=1024)
            y_drain(*RES_W, "v")

            om_pair(0, 0, "v")
            om_pair(0, 1, "a")
            om_pair(1, 0, "v")

            # rest: (0,2),(0,3) pair in freed psM (merged drain), then
            # (1,1) psW, (1,2) psG, (1,3) psW
            yM2 = psM.tile([128, 1024], F32, tag="m", name="yM2")
            y_ps[(0, 2)] = yM2[:, 0:512]
            y_ps[(0, 3)] = yM2[:, 512:1024]
            for (st, nb) in ((0, 2), (0, 3)):
                for p in range(4):
                    y_step(st, nb, p)
            y_ps.pop((0, 3))
            y_ps[(0, 2)] = yM2
            y_drain(0, 2, "a", width=1024)
            om_pair(0, 2, "v")
            om_pair(0, 3, "a")

            y_rot = [psW, psG, psW]
            for i, (st, nb) in enumerate([(1, 1), (1, 2), (1, 3)]):
                pool = y_rot.pop(0)
                y_ps[(st, nb)] = pool.tile(
                    [128, 512], F32, tag="w" if pool is psW else "g",
                    name=f"y{st}{nb}")
                for p in range(4):
                    y_step(st, nb, p)
                y_drain(st, nb, "v")
                om_pair(st, nb, "a" if i % 2 == 0 else "v")

    nc.compile()
    return nc


def _get_nc():
    if "nc" not in _NC_CACHE:
        _NC_CACHE["nc"] = build_nc()
    return _NC_CACHE["nc"]


def _prep_inputs(A, B, W_phi, W_theta, W_g, W_AB, W_mask):
    A = np.ascontiguousarray(np.asarray(A, np.float32)).reshape(4, 256, 4096)
    B = np.ascontiguousarray(np.asarray(B, np.float32)).reshape(4, 256, 4096)
    W_phi = np.asarray(W_phi, np.float32)
    W_theta = np.asarray(W_theta, np.float32)
    W_g = np.asarray(W_g, np.float32)
    W_AB = np.asarray(W_AB, np.float32)
    W_mask = np.asarray(W_mask, np.float32)

    def q8(x):
        return np.clip(x, -240.0, 240.0).astype(E4NP)

    A8 = q8(A)
    B8 = q8(B)
    Alo = q8((A - A8.astype(np.float32)) * 16.0)
    Blo = q8((B - B8.astype(np.float32)) * 16.0)

    def chansplit(w):
        # (ch_out, 256) -> [128 icl, 2 ich, ch_out]
        t = w.T.reshape(2, 128, -1)  # (ich, icl, ch)
        return t.transpose(1, 0, 2)

    wq1 = np.zeros((128, 10, 128), np.float32)
    wq1[:, 0:2, :] = chansplit(W_theta * 64.0)
    wq1[:, 2:4, :] = chansplit(W_phi * 64.0)
    wg = (W_g * 64.0).T.reshape(4, 128, 128).transpose(1, 0, 2)  # [jl, jc, chg]
    wq1[:, 4:6, :] = wg[:, 0:2, :]
    wq1[:, 6:8, :] = wg[:, 2:4, :]
    wmkT = (W_mask * 4.0).T  # (128 i, 256 oc)
    wq1[:, 8, :] = wmkT[:, 0:128]
    wq1[:, 9, :] = wmkT[:, 128:256]
    wq1 = np.ascontiguousarray(q8(wq1).reshape(128, 1280))

    Whi = q8(W_AB * 64.0)
    Wlo = q8((W_AB * 64.0 - Whi.astype(np.float32)) * 16.0)
    terms = [Whi.astype(np.float32), Whi.astype(np.float32) / 16.0,
             Wlo.astype(np.float32) / 16.0]
    wq2 = np.zeros((128, 12, 2, 128), np.float32)
    for t_i, T in enumerate(terms):
        # T: (256 oc, 512 j) -> lhsT [jl, jh, oc] per (och, jstep)
        for och in range(2):
            for jstep in range(2):
                blk = T[128 * och:128 * (och + 1),
                        256 * jstep:256 * (jstep + 1)]  # (128 oc, 256 j)
                wq2[:, t_i * 4 + och * 2 + jstep, :, :] = (
                    blk.T.reshape(2, 128, 128).transpose(1, 0, 2))
    wq2 = np.ascontiguousarray(q8(wq2).reshape(128, 3072))

    in_maps = []
    for core in range(8):
        b, h = core // 2, core % 2
        scols = np.r_[1024 * h:1024 * (h + 1),
                      2048 + 1024 * h:2048 + 1024 * (h + 1)]

        def strip3(x):
            return np.ascontiguousarray(
                x[:, scols].reshape(2, 128, 2048).transpose(1, 0, 2))

        b8q = B8[b].reshape(2, 128, 4, 1024)[:, :, [0, 2, 1, 3], :]
        in_maps.append({
            "B8": np.ascontiguousarray(
                b8q.transpose(1, 0, 2, 3).reshape(128, 2, 2, 2048)),
            "AH8": strip3(A8[b]),
            "BH8": strip3(B8[b]),
            "ALO": strip3(Alo[b]),
            "BLO": strip3(Blo[b]),
            "WQ1": wq1,
            "WQ2": wq2,
        })
    return in_maps


def _combine(results):
    out = np.zeros((4, 256, 4096), dtype=np.float32)
    for core in range(8):
        b, h = core // 2, core % 2
        om = results[core]["OM8"].astype(np.float32)  # [128, 2 st, 2 och, 2048]
        om = om.transpose(2, 0, 1, 3).reshape(256, 4096)
        out[b] += om * (2.0 ** -9)
        ow = results[core]["OW"].astype(np.float32).reshape(256, 2048)
        out[b][:, 1024 * h:1024 * (h + 1)] += ow[:, 0:1024]
        out[b][:, 2048 + 1024 * h:2048 + 1024 * (h + 1)] += ow[:, 1024:2048]
    return out.reshape(4, 256, 64, 64)


def run(inputs, **kwargs):
    nc = _get_nc()
    in_maps = _prep_inputs(**inputs)
    try:
        res = run_bass_kernel_spmd(nc, in_maps, core_ids=list(range(8)), **kwargs)
    except Exception:
        # transient NRT device wedge: retry once
        res = run_bass_kernel_spmd(nc, in_maps, core_ids=list(range(8)), **kwargs)
    return _combine(res.results), res


def kernel(A, B, W_phi, W_theta, W_g, W_AB, W_mask):
    out, _ = run(dict(A=A, B=B, W_phi=W_phi, W_theta=W_theta, W_g=W_g,
                      W_AB=W_AB, W_mask=W_mask))
    return out


if __name__ == "__main__":
    rng = np.random.default_rng(0)
    ins = {
        "A": rng.standard_normal((4, 256, 64, 64)).astype(np.float32),
        "B": rng.standard_normal((4, 256, 64, 64)).astype(np.float32),
        "W_phi": (rng.standard_normal((128, 256)) * 0.02).astype(np.float32),
        "W_theta": (rng.standard_normal((128, 256)) * 0.02).astype(np.float32),
        "W_g": (rng.standard_normal((128, 512)) * 0.02).astype(np.float32),
        "W_AB": (rng.standard_normal((256, 512)) * 0.02).astype(np.float32),
        "W_mask": (rng.standard_normal((256, 128)) * 0.02).astype(np.float32),
    }
    out = kernel(**ins)
    print("kernel out", out.shape, out.dtype, float(np.abs(out).max()))
